# revision 1
# baseline (speedup 1.0000x reference)
"""Binary 3x3 conv (XNOR-net style) on 8 Trainium2 NeuronCores.

out = alpha * (sign(x) conv sign(w)), NHWC, SAME padding.
Data-parallel over batch: each of the 8 cores handles 8 images.

Per-core pipeline (all layout math hardcoded for x=(64,128,128,64) fp32):
  1. SWDGE cast-DMA image (fp32 HBM -> bf16 SBUF), row-major [row, w*64+ci],
     with 64-elem zero pads on both ends of each row.
  2. ACT Sign -> +-1 bf16 (exact in bf16).
  3. HWDGE xbar-transpose DMA -> "layout B": [k=channel-of-staggered-pixel-pair,
     (pair c, row r)] where k<64 is x[2c-1] channels, k>=64 is x[2c] channels.
  4. TensorE: 6 matmuls (K=128, M=128, N<=512) per 8 output rows, accumulating
     integer +-1 counts into one PSUM bank.  M packs (even-pixel cout | odd-pixel
     cout); weights are host-packed sign matrices with zero quadrants.
  5. ACT Copy evict PSUM fp32 counts -> fp16 SBUF (counts <= 576, exact).
  6. HWDGE xbar-transpose back to pixel-major fp16.
  7. DVE tensor_mul with replicated fp32 alpha -> fp32.
  8. Contiguous DMA store.
"""

import os
import sys

sys.path.insert(0, "/opt/trn_rl_repo")

import numpy as np
import ml_dtypes

import concourse.bass as bass
import concourse.mybir as mybir
from concourse.tile import TileContext
from concourse.bass_utils import run_bass_kernel_spmd

N_CORES = 8
IMGS_PER_CORE = 8
H = W = 128
C = 64
ROW = W * C          # 8192 elems per image row
XPAD = 64            # one pixel of zero padding each side
XROW = ROW + 2 * XPAD  # 8320 = 65*128


def _split_multi_waits(nc):
    """The walrus in this container allows only ONE sync-wait per instruction.

    Tile attaches several waits to some instructions; hoist the extras onto
    single-wait NOPs inserted immediately before, on the same engine (the
    engine blocks on each in program order, so semantics are unchanged)."""
    n_new = 0
    for f in nc.m.functions:
        for bb in f.blocks:
            insts = bb.instructions
            if not any(
                i.sync_info is not None and len(i.sync_info.on_wait) > 1
                for i in insts
            ):
                continue
            new = []
            for inst in insts:
                si = inst.sync_info
                if si is not None and len(si.on_wait) > 1:
                    waits = list(si.on_wait)
                    for j, w in enumerate(waits[:-1]):
                        n_new += 1
                        new.append(mybir.InstNoOp(
                            name=f"{inst.name}-sw{j}",
                            engine=inst.engine,
                            bass_nofuse=True,
                            sync_info=mybir.SyncInfo(on_wait=[w], on_update=[]),
                        ))
                    si.on_wait.clear()
                    si.on_wait.append(waits[-1])
                new.append(inst)
            bb.instructions = new
    return n_new


def _pack_weights(w_fp: np.ndarray):
    """Host-side weight prep: sign matrices A/B per kh, and alpha replication."""
    ws = np.where(w_fp >= 0, 1.0, -1.0).astype(np.float32)  # (kh, kw, ci, co)
    wst = np.zeros((6, 128, 128), np.float32)
    for kh in range(3):
        A = wst[2 * kh]
        B = wst[2 * kh + 1]
        # M columns: m<64 -> even out pixel w=2g cout m ; m>=64 -> odd w=2g+1.
        # K rows: k<64 -> x[2g-1] chan k ; k>=64 -> x[2g] chan k-64 (rhs pair g)
        # B variant reads pair g+1: k<64 -> x[2g+1], k>=64 -> x[2g+2].
        A[0:64, 0:64] = ws[kh, 0]
        A[64:128, 0:64] = ws[kh, 1]
        A[64:128, 64:128] = ws[kh, 0]
        B[0:64, 0:64] = ws[kh, 2]
        B[0:64, 64:128] = ws[kh, 1]
        B[64:128, 64:128] = ws[kh, 2]
    alpha = np.mean(np.abs(w_fp), axis=(0, 1, 2)).astype(np.float32)  # (co,)
    alpha_rep = np.tile(alpha, 32)[None, :].repeat(128, axis=0)  # (128, 2048)
    return wst.astype(ml_dtypes.bfloat16), np.ascontiguousarray(alpha_rep)


_PROGRAM_CACHE = {}


def _build_program(repeats: int = 1, skip: tuple = ()):
    key = (repeats, tuple(sorted(skip)))
    if key in _PROGRAM_CACHE:
        return _PROGRAM_CACHE[key]
    skip = set(skip)

    f32 = mybir.dt.float32
    f16 = mybir.dt.float16
    bf16 = mybir.dt.bfloat16
    Copy = mybir.ActivationFunctionType.Copy

    nc = bass.Bass()
    x_d = nc.dram_tensor("x", (IMGS_PER_CORE, H, W, C), f32, kind="ExternalInput")
    wst_d = nc.dram_tensor("wst", (6, 128, 128), bf16, kind="ExternalInput")
    al_d = nc.dram_tensor("alpha_rep", (128, 2048), f32, kind="ExternalInput")
    out_d = nc.dram_tensor("out", (IMGS_PER_CORE, H, W, C), f32, kind="ExternalOutput")

    x_flat = x_d.rearrange("i h w c -> i (h w c)")      # [8, 1048576]
    out_flat = out_d.rearrange("i h w c -> i (h w c)")  # [8, 1048576]

    with TileContext(nc) as tc:
        with (
            tc.tile_pool(name="wpool", bufs=1) as wpool,
            tc.tile_pool(name="xpool", bufs=2) as xpool,
            tc.tile_pool(name="xtpool", bufs=2) as xtpool,
            tc.tile_pool(name="ppool", bufs=8, space="PSUM") as ppool,
            tc.tile_pool(name="opool", bufs=3) as opool,
            tc.tile_pool(name="tpool", bufs=3) as tpool,
            tc.tile_pool(name="fpool", bufs=3) as fpool,
        ):
            wst_sb = wpool.tile([128, 6 * 128], bf16)
            nc.sync.dma_start(out=wst_sb.rearrange("k (i m) -> k i m", m=128),
                              in_=wst_d.rearrange("i k m -> k i m"))
            alpha_sb = wpool.tile([128, 2048], f32)
            nc.sync.dma_start(out=alpha_sb[:], in_=al_d[:])
            if skip:
                XCONST = wpool.tile([128, XROW], bf16)
                nc.vector.memset(XCONST[:, 0:XROW], 0.0)
                XTCONST = wpool.tile([128, XROW], bf16)
                nc.vector.memset(XTCONST[:, 0:XROW], 0.0)
                O4CONST = wpool.tile([128, 2048], f16)
                nc.vector.memset(O4CONST[:, 0:2048], 0.0)
                T2CONST = wpool.tile([128, 2048], f16)
                nc.vector.memset(T2CONST[:, 0:2048], 0.0)
                F32CONST = wpool.tile([128, 2048], f32)
                nc.vector.memset(F32CONST[:, 0:2048], 0.0)

            for img_rep in range(IMGS_PER_CORE * repeats):
                img = img_rep % IMGS_PER_CORE
                # --- load + sign + transpose to layout B ---
                if "cast" not in skip:
                    X = xpool.tile([128, XROW], bf16, tag="X")
                    nc.vector.memset(X[:, 0:XPAD], 0.0)
                    nc.vector.memset(X[:, XPAD + ROW:], 0.0)
                    nc.gpsimd.dma_start(
                        out=X[:, XPAD:XPAD + ROW],
                        in_=x_flat[img].rearrange("(h i) -> h i", h=128),
                    )
                    if "sign" not in skip:
                        nc.scalar.sign(X[:, XPAD:XPAD + ROW], X[:, XPAD:XPAD + ROW])
                else:
                    X = XCONST
                if "inxbar" not in skip:
                    XT = xtpool.tile([128, XROW], bf16, tag="XT")
                    XT3 = XT.rearrange("q (c r) -> q c r", r=128)
                    # split into 8 transposes: concurrent xbar DMAs aggregate to
                    # ~375 GB/s vs ~107 GB/s for one monolithic call (HW-measured)
                    for i8 in range(8):
                        c0 = i8 * 8
                        cn = 8 if i8 < 7 else 9
                        nc.sync.dma_start(
                            out=XT3[:, c0:c0 + cn, :],
                            in_=X[:, c0 * 128:(c0 + cn) * 128],
                            transpose=True,
                        )
                else:
                    XT = XTCONST
                # [128, c=65, r=128]; matmul rhs iterates (c outer, r inner) so the
                # innermost stream dim is stride-1 (16B runs) — strided innermost
                # dims run the PE ~3x slower (HW-measured).
                XT_cr = XT.rearrange("q (c r) -> q c r", r=128)

                # --- conv blocks: 16 blocks of 8 output rows; groups of 4 ---
                for grp in range(4):
                    if "mm" not in skip:
                        O4 = opool.tile([128, 2048], f16, tag="O4")
                    else:
                        O4 = O4CONST
                    for blk in range(4) if "mm" not in skip else []:
                        h0 = grp * 32 + blk * 8
                        psum = ppool.tile([128, 512], f32, tag="ps")
                        # same addresses (n = r*64 + c) but iterated (c outer,
                        # r inner) to pair with the rhs stream order
                        psum_cr = psum.rearrange("p (r c) -> p c r", c=64)
                        mms = []
                        for kh in (1, 0, 2):
                            rbase = h0 + kh - 1
                            r_lo = max(0, -rbase)
                            r_hi = min(8, 128 - rbase)
                            for v in (0, 1):  # A, B
                                mms.append((kh, v, rbase, r_lo, r_hi))
                        last = len(mms) - 1
                        for idx, (kh, v, rbase, r_lo, r_hi) in enumerate(mms):
                            lhsT = wst_sb[:, (2 * kh + v) * 128:(2 * kh + v + 1) * 128]
                            rhs = XT_cr[:, v:v + 64, rbase + r_lo:rbase + r_hi]
                            outp = psum_cr[:, :, r_lo:r_hi]
                            nc.tensor.matmul(outp, lhsT, rhs,
                                             start=(idx == 0), stop=(idx == last))
                        nc.scalar.activation(out=O4[:, blk * 512:(blk + 1) * 512],
                                             in_=psum[:], func=Copy)
                    # --- transpose back, scale, store (32 rows = 262144 elems) ---
                    if "oxbar" not in skip:
                        T2 = tpool.tile([128, 2048], f16, tag="T2")
                        nc.sync.dma_start(
                            out=T2.rearrange("q (m p) -> q m p", p=128),
                            in_=O4[:],
                            transpose=True,
                        )
                    else:
                        T2 = T2CONST
                    if "tt" not in skip:
                        F32 = fpool.tile([128, 2048], f32, tag="F32")
                        nc.vector.tensor_mul(out=F32[:], in0=T2[:], in1=alpha_sb[:])
                    else:
                        F32 = F32CONST
                    if "store" not in skip:
                        dst = out_flat[img, grp * 262144:(grp + 1) * 262144]
                        # SWDGE ring: keeps the 128us of store traffic off the
                        # SP HWDGE ring where it serialized with the xbar
                        # transposes (safe: SBUF->DRAM copy, no xbar mode).
                        nc.gpsimd.dma_start(
                            out=dst.rearrange("(m q p) -> q m p", m=16, q=128, p=128),
                            in_=F32.rearrange("q (m p) -> q m p", p=128),
                        )

    _split_multi_waits(nc)
    _PROGRAM_CACHE[key] = nc
    return nc


def kernel(x: np.ndarray, w_fp: np.ndarray) -> np.ndarray:
    assert x.shape == (64, 128, 128, 64) and w_fp.shape == (3, 3, 64, 64)
    x = np.ascontiguousarray(x, dtype=np.float32)
    wst, alpha_rep = _pack_weights(np.asarray(w_fp, dtype=np.float32))

    nc = _build_program()
    in_maps = []
    for i in range(N_CORES):
        in_maps.append({
            "x": x[i * IMGS_PER_CORE:(i + 1) * IMGS_PER_CORE],
            "wst": wst,
            "alpha_rep": alpha_rep,
        })
    res = run_bass_kernel_spmd(nc, in_maps, core_ids=list(range(N_CORES)))
    out = np.concatenate([r["out"] for r in res.results], axis=0)
    # stash perf info for test harnesses
    kernel.last_results = res
    return out

